# revision 16
# baseline (speedup 1.0000x reference)
"""TRN2 Bass kernel for nn_MLA_87892210746097.

MHA with RoPE, double softmax, o_proj. B=2, S=2048, E=2048, H=16, D=128.
Sharding: 8 cores = 2 batches x 4 head-groups (4 heads each); the o_proj
all-reduce (4 partial sums per batch) and bias add happen on the host after
the gather, as does the head-axis concat of the attention weights.
Returns (out, attention_weights) like the reference.
"""
import sys
import numpy as np
import ml_dtypes
from contextlib import ExitStack

sys.path.insert(0, "/opt/trn_rl_repo")

import concourse.bass as bass
import concourse.mybir as mybir
import concourse.tile as tile
from concourse import bacc
from concourse.bass_utils import run_bass_kernel_spmd

F32 = mybir.dt.float32
F32R = mybir.dt.float32r
F16 = mybir.dt.float16
FP8 = mybir.dt.float8e5
AF = mybir.ActivationFunctionType
ALU = mybir.AluOpType

B, S, E = 2, 2048, 2048
H, D = 16, 128
HPC = 4            # heads per core
DL = HPC * D       # 512: local E-slice per core
NKC = S // 128     # 16 chunks of 128
NTS = S // 512     # 4 slices of 512
NEC = E // 128     # 16 e-chunks
SCALE = 1.0 / float(np.sqrt(D))
BIG = 57344.0      # fp8e5-exact large value for the mask seed

_NC_CACHE = None


def _swap_matrix():
    """lhsT for rot = M @ q^T where M is the minus_swap permutation."""
    Mt = np.zeros((D, D), dtype=np.float32)
    for i in range(D // 2):
        Mt[2 * i + 1, 2 * i] = -1.0   # out[2i]   = -q[2i+1]
        Mt[2 * i, 2 * i + 1] = 1.0    # out[2i+1] =  q[2i]
    return Mt


def build_kernel():
    nc = bacc.Bacc(None, target_bir_lowering=False)

    # ---------------- I/O ----------------
    xT = nc.declare_dram_parameter("xT", [E, S], F32R, isOutput=False)
    wq = nc.declare_dram_parameter("wq", [E, DL], F32R, isOutput=False)
    wk = nc.declare_dram_parameter("wk", [E, DL], F32R, isOutput=False)
    wv = nc.declare_dram_parameter("wv", [E, DL], F32R, isOutput=False)
    wo = nc.declare_dram_parameter("wo", [DL, E], F32R, isOutput=False)
    bq = nc.declare_dram_parameter("bq", [1, DL], F32, isOutput=False)
    bk = nc.declare_dram_parameter("bk", [1, DL], F32, isOutput=False)
    bv = nc.declare_dram_parameter("bv", [1, DL], F32, isOutput=False)
    sinT = nc.declare_dram_parameter("sinT", [HPC, D, S], F32, isOutput=False)
    cosT = nc.declare_dram_parameter("cosT", [HPC, D, S], F32, isOutput=False)
    # host-prepped (mask.T - 1) in fp8e5: 0 where visible, -1 where masked
    mm1T = nc.declare_dram_parameter("mm1T", [S, S], FP8, isOutput=False)

    out_p = nc.declare_dram_parameter("out_p", [S, E], F32, isOutput=True)
    w_out = nc.declare_dram_parameter("w_out", [HPC, S, S], F32, isOutput=True)

    # ---------------- constants ----------------
    swapM = nc.inline_tensor(_swap_matrix(), name="swapM")
    seedI = nc.inline_tensor(
        (np.eye(128) * BIG).astype(np.float16), name="seedI")
    ones128 = nc.inline_tensor(np.ones((128, 1), dtype=np.float16), name="ones128")
    identF = nc.inline_tensor(np.eye(128, dtype=np.float32), name="identF")
    ident16 = nc.inline_tensor(np.eye(128).astype(np.float16), name="ident16")

    # ---------------- scratch ----------------
    qT_s = nc.dram_tensor("qT_s", [HPC, D, S], F32R)
    kT_s = nc.dram_tensor("kT_s", [HPC, D, S], F32R)
    v_s = nc.dram_tensor("v_s", [S, DL], F16)
    attnT_s = nc.dram_tensor("attnT_s", [HPC, D, S], F32R)
    r_s = nc.dram_tensor("r_s", [1, S], F32)
    r2_s = nc.dram_tensor("r2_s", [1, S], F32)

    with ExitStack() as ctx:
        tc = ctx.enter_context(tile.TileContext(nc))
        consts = ctx.enter_context(tc.tile_pool(name="consts", bufs=1))
        big = ctx.enter_context(tc.tile_pool(name="big", bufs=1))
        work = ctx.enter_context(tc.tile_pool(name="work", bufs=1))
        reps = ctx.enter_context(tc.tile_pool(name="reps", bufs=1))
        ps = ctx.enter_context(tc.tile_pool(name="ps", bufs=1, space="PSUM"))

        def ps_a(shape, name):
            return ps.tile(shape, F32, tag="a", bufs=2, name=name)

        def ps_b(shape, name):
            return ps.tile(shape, F32, tag="b", bufs=2, name=name)

        def ps_d(shape, name):
            return ps.tile(shape, F32, tag="d", bufs=1, name=name)

        swap_sb = consts.tile([D, D], F32R)
        nc.sync.dma_start(out=swap_sb, in_=swapM[:].bitcast(F32R))
        seed16 = consts.tile([128, 128], F16)
        nc.sync.dma_start(out=seed16, in_=seedI[:])
        seed_sb = consts.tile([128, 128], FP8)
        nc.vector.tensor_copy(out=seed_sb, in_=seed16)
        ones128_sb = consts.tile([128, 1], F16)
        nc.sync.dma_start(out=ones128_sb, in_=ones128[:])
        ident_sb = consts.tile([128, 128], F32)
        nc.sync.dma_start(out=ident_sb, in_=identF[:])
        ident16_sb = consts.tile([128, 128], F16)
        nc.sync.dma_start(out=ident16_sb, in_=ident16[:])
        # per-partition bias views: (p, c) <- b[0, c*128 + p]
        bqt = consts.tile([128, HPC], F32)
        nc.sync.dma_start(out=bqt, in_=bq[0].rearrange("(c p) -> p c", p=128))
        bkt = consts.tile([128, HPC], F32)
        nc.sync.dma_start(out=bkt, in_=bk[0].rearrange("(c p) -> p c", p=128))
        # bv broadcast to 128 partitions (free-axis bias add on V)
        bv_rep = consts.tile([128, DL], F32)
        bvap = bv[:]
        nc.gpsimd.dma_start(
            out=bv_rep,
            in_=bass.AP(tensor=bvap.tensor, offset=bvap.offset,
                        ap=[[0, 128], bvap.ap[-1]]))

        # ============ big slab tags (8 x 16KB/partition) ============
        # tA/tB: wq halves -> pw slabs 0/1 -> aT_all halves
        # tC/tD: wk halves -> pw slabs 2/3
        # tE/tF: wv halves -> wo halves
        # tG/tH: x-slice halves
        # sq: qT -> vloc ; sk: kT -> values
        def big_tile(shape, dtype, tag, name):
            return big.tile(shape, dtype, tag=tag, bufs=1, name=name)

        wq_t = [big_tile([128, 8, DL], F32R, t, f"wqh{i}")
                for i, t in enumerate(("tA", "tB"))]
        wk_t = [big_tile([128, 8, DL], F32R, t, f"wkh{i}")
                for i, t in enumerate(("tC", "tD"))]
        wv_t = [big_tile([128, 8, DL], F32R, t, f"wvh{i}")
                for i, t in enumerate(("tE", "tF"))]
        for i in range(2):
            half = slice(i * 8, (i + 1) * 8)
            nc.sync.dma_start(
                out=wq_t[i], in_=wq.rearrange("(n p) d -> p n d", p=128)[:, half])
            nc.sync.dma_start(
                out=wk_t[i], in_=wk.rearrange("(n p) d -> p n d", p=128)[:, half])
            nc.sync.dma_start(
                out=wv_t[i], in_=wv.rearrange("(n p) d -> p n d", p=128)[:, half])

        def wslab(tiles, ec):
            return tiles[ec // 8][:, ec % 8, :]

        # ================= projection phase =================
        for ts_i in range(NTS):
            tsl = slice(ts_i * 512, (ts_i + 1) * 512)
            x_t = [big_tile([128, 8, 512], F32R, t, f"xh{i}_{ts_i}")
                   for i, t in enumerate(("tG", "tH"))]
            for i in range(2):
                half = slice(i * 8, (i + 1) * 8)
                nc.sync.dma_start(
                    out=x_t[i],
                    in_=xT.rearrange("(n p) s -> p n s", p=128)[:, half, tsl])

            # --- q^T and k^T (d on partitions) + RoPE ---
            for w_t, bias_t, dst in ((wq_t, bqt, qT_s), (wk_t, bkt, kT_s)):
                for oc in range(HPC):  # head == o-chunk of 128
                    pj_ps = ps_a([128, 512], f"pj{ts_i}{oc}")
                    for ec in range(NEC):
                        nc.tensor.matmul(
                            pj_ps, wslab(w_t, ec)[:, oc * 128:(oc + 1) * 128],
                            wslab(x_t, ec), start=(ec == 0), stop=(ec == NEC - 1))
                    qraw = work.tile([128, 512], F32, tag="qraw", name="qraw")
                    nc.scalar.activation(out=qraw, in_=pj_ps, func=AF.Identity,
                                         bias=bias_t[:, oc:oc + 1], scale=1.0)
                    qraw_r = work.tile([128, 512], F32R, tag="qrawr", name="qraw_r")
                    nc.vector.tensor_copy(out=qraw_r, in_=qraw)
                    rot_ps = ps_b([128, 512], f"rot{ts_i}{oc}")
                    nc.tensor.matmul(rot_ps, swap_sb, qraw_r, start=True, stop=True)
                    sin_t = work.tile([128, 512], F32, tag="sin", name="sin_t")
                    cos_t = work.tile([128, 512], F32, tag="cos", name="cos_t")
                    nc.sync.dma_start(out=sin_t, in_=sinT[oc, :, tsl])
                    nc.sync.dma_start(out=cos_t, in_=cosT[oc, :, tsl])
                    t1 = work.tile([128, 512], F32, tag="t1", name="t1")
                    nc.vector.tensor_tensor(t1, qraw, cos_t, ALU.mult)
                    t2 = work.tile([128, 512], F32, tag="t2", name="t2")
                    nc.vector.tensor_tensor(t2, rot_ps, sin_t, ALU.mult)
                    qrot = work.tile([128, 512], F32R, tag="qrawr", name="qrot")
                    nc.vector.tensor_tensor(qrot, t1, t2, ALU.add)
                    nc.sync.dma_start(out=dst[oc, :, tsl], in_=qrot)

            # --- V (tokens on partitions) ---
            for tci in range(4):
                tchunk = ts_i * 4 + tci
                v_ps = ps_a([128, DL], f"vps{tchunk}")
                for ec in range(NEC):
                    nc.tensor.matmul(
                        v_ps, wslab(x_t, ec)[:, tci * 128:(tci + 1) * 128],
                        wslab(wv_t, ec), start=(ec == 0), stop=(ec == NEC - 1))
                v_sb = work.tile([128, DL], F16, tag="vsb", bufs=2, name="v_sb")
                nc.vector.tensor_tensor(v_sb, v_ps, bv_rep, ALU.add)
                nc.sync.dma_start(out=v_s[tchunk * 128:(tchunk + 1) * 128, :],
                                  in_=v_sb)

        # ================= attention per head =================
        for h in range(HPC):
            qT_sb = big_tile([D, S], F32R, "sq", f"qT{h}")
            kT_sb = big_tile([D, S], F32R, "sk", f"kT{h}")
            nc.sync.dma_start(out=qT_sb, in_=qT_s[h])
            nc.sync.dma_start(out=kT_sb, in_=kT_s[h])
            pw_t = [big_tile([128, 4, S], F16, t, f"pw{i}_{h}")
                    for i, t in enumerate(("tA", "tB", "tC", "tD"))]

            def pw(kc):
                return pw_t[kc // 4][:, kc % 4, :]

            # ---- pass 1: logits -> exp -> P^T, rowsum r (partition dir) ----
            r_q = [ps.tile([1, 512], F32, tag=t, bufs=2, name=f"rq{j}_{h}")
                   for j, t in enumerate(("a", "a", "b", "b"))]
            for kc in range(NKC):
                mk = work.tile([128, S], FP8, tag="mk", bufs=2, name="mk")
                nc.sync.dma_start(out=mk, in_=mm1T[kc * 128:(kc + 1) * 128, :])
                for hf in range(2):
                    hsl_ = slice(hf * 1024, (hf + 1) * 1024)
                    lg_ps = ps.tile([128, 1024], F32, tag="d", bufs=2,
                                    name=f"lg{h}{kc}{hf}")
                    for j in range(2):
                        jsl = slice(hf * 1024 + j * 512, hf * 1024 + (j + 1) * 512)
                        jloc = slice(j * 512, (j + 1) * 512)
                        nc.tensor.matmul(lg_ps[:, jloc], seed_sb, mk[:, jsl],
                                         start=True, stop=False)
                        nc.tensor.matmul(lg_ps[:, jloc],
                                         kT_sb[:, kc * 128:(kc + 1) * 128],
                                         qT_sb[:, jsl], start=False, stop=True)
                    nc.scalar.activation(out=pw(kc)[:, hsl_], in_=lg_ps,
                                         func=AF.Exp, bias=0.0, scale=SCALE)
                for j in range(NTS):
                    jsl = slice(j * 512, (j + 1) * 512)
                    nc.tensor.matmul(r_q[j], ones128_sb, pw(kc)[:, jsl],
                                     start=(kc == 0), stop=(kc == NKC - 1))

            # 1/r -> dram roundtrip; fp16 bcast rep + per-partition chunks
            rrecf = reps.tile([1, S], F32, tag="rrecf", name="rrecf")
            for j in range(NTS):
                nc.vector.reciprocal(out=rrecf[:, j * 512:(j + 1) * 512],
                                     in_=r_q[j])
            nc.sync.dma_start(out=r_s[:], in_=rrecf)
            rpp = reps.tile([128, NKC], F32, tag="rpp", name="rpp")
            nc.sync.dma_start(out=rpp, in_=r_s[0].rearrange("(c p) -> p c", p=128))
            rrep16 = reps.tile([128, S], F16, tag="rrep16", name="rrep16")
            rsap = r_s[:]
            nc.gpsimd.dma_start(
                out=rrep16,
                in_=bass.AP(tensor=rsap.tensor, offset=rsap.offset,
                            ap=[[0, 128], rsap.ap[-1]]))

            # ---- pass 2: values accum, A^T = P^T/r, exp2 -> Wun, rowsum2 ----
            vloc = big_tile([128, NKC, D], F16, "sq", f"vloc{h}")
            nc.sync.dma_start(
                out=vloc,
                in_=v_s.rearrange("(n p) d -> p n d", p=128)[:, :, h * D:(h + 1) * D])
            valT_h = [ps.tile([128, 1024], F32, tag="d", bufs=2,
                              name=f"valT{h}{i}") for i in range(2)]
            r2_q = [ps.tile([1, 512], F32, tag=t, bufs=2, name=f"r2q{j}_{h}")
                    for j, t in enumerate(("a", "a", "b", "b"))]
            for kc in range(NKC):
                for j in range(NTS):
                    jsl = slice(j * 512, (j + 1) * 512)
                    nc.tensor.matmul(
                        valT_h[j // 2][:, (j % 2) * 512:(j % 2 + 1) * 512],
                        vloc[:, kc, :], pw(kc)[:, jsl],
                        start=(kc == 0), stop=(kc == NKC - 1))
                aT = work.tile([128, S], F16, tag="aT", bufs=2, name="aT")
                nc.vector.tensor_tensor(aT, pw(kc), rrep16, ALU.mult)
                nc.scalar.activation(out=pw(kc), in_=aT,
                                     func=AF.Exp, bias=0.0, scale=1.0)
                for j in range(NTS):
                    jsl = slice(j * 512, (j + 1) * 512)
                    nc.tensor.matmul(r2_q[j], ones128_sb, pw(kc)[:, jsl],
                                     start=(kc == 0), stop=(kc == NKC - 1))

            # values^T = valT/r (free-axis scale) -> fp16, then PE-transpose
            # to (s, d) layout for the pass-3 stationary
            valuesT = work.tile([D, S], F16, tag="vT", name="valuesT")
            for i in range(2):
                nc.vector.tensor_tensor(valuesT[:, i * 1024:(i + 1) * 1024],
                                        valT_h[i], rrep16[:, i * 1024:(i + 1) * 1024],
                                        ALU.mult)
            values = big_tile([128, NKC, D], F16, "sk", f"values{h}")
            for sc in range(NKC):
                vt_ps = ps_a([128, 128], f"vt{h}{sc}") if sc % 2 == 0 else \
                    ps_b([128, 128], f"vt{h}{sc}")
                vt_ps = vt_ps.bitcast(F16)[:, 0:128]
                nc.tensor.transpose(
                    vt_ps, valuesT[:, sc * 128:(sc + 1) * 128], ident16_sb)
                nc.scalar.copy(out=values[:, sc, :], in_=vt_ps)

            # 1/r2 -> dram roundtrip -> fp16 broadcast rep
            r2recf = reps.tile([1, S], F32, tag="rrecf", name="r2recf")
            for j in range(NTS):
                nc.vector.reciprocal(out=r2recf[:, j * 512:(j + 1) * 512],
                                     in_=r2_q[j])
            nc.sync.dma_start(out=r2_s[:], in_=r2recf)
            r2rep16 = reps.tile([128, S], F16, tag="r2rep16", name="r2rep16")
            r2ap = r2_s[:]
            nc.gpsimd.dma_start(
                out=r2rep16,
                in_=bass.AP(tensor=r2ap.tensor, offset=r2ap.offset,
                            ap=[[0, 128], r2ap.ap[-1]]))

            # ---- pass 3a: attn_un^T accum over k, scale by 1/r2 ----
            at_h = [ps.tile([128, 1024], F32, tag="d", bufs=2,
                            name=f"atps{h}{i}") for i in range(2)]
            for kc in range(NKC):
                for j in range(NTS):
                    jsl = slice(j * 512, (j + 1) * 512)
                    nc.tensor.matmul(
                        at_h[j // 2][:, (j % 2) * 512:(j % 2 + 1) * 512],
                        values[:, kc, :], pw(kc)[:, jsl],
                        start=(kc == 0), stop=(kc == NKC - 1))
            for j in range(NTS):
                jsl = slice(j * 512, (j + 1) * 512)
                attnT = work.tile([128, 512], F32R, tag="osb", bufs=2,
                                  name="attnT")
                nc.vector.tensor_tensor(
                    attnT, at_h[j // 2][:, (j % 2) * 512:(j % 2 + 1) * 512],
                    r2rep16[:, jsl], ALU.mult)
                nc.sync.dma_start(out=attnT_s[h][:, jsl], in_=attnT)

            # ---- pass 3b: W output: scale, transpose, write (q,k) rows ----
            # scale slab in place (W^T = Wun^T / r2), row-wise fp16
            for kc in range(NKC):
                nc.vector.tensor_tensor(pw(kc), pw(kc), r2rep16, ALU.mult)
            for qb in range(NKC):
                wrow = work.tile([128, S], F32, tag="qraw", name="wrow")
                for hf in range(2):
                    tp_ps = ps.tile([128, 1024], F32, tag="d", bufs=2,
                                    name=f"tp{h}{qb}{hf}")
                    tp16 = tp_ps.bitcast(F16)[:, 0:1024]
                    for kk in range(8):
                        kc = hf * 8 + kk
                        nc.tensor.matmul(
                            tp16[:, kk * 128:(kk + 1) * 128],
                            pw(kc)[:, qb * 128:(qb + 1) * 128], ident16_sb,
                            is_transpose=True,
                            start=(kk == 0), stop=(kk == 7))
                    nc.any.tensor_copy(
                        out=wrow[:, hf * 1024:(hf + 1) * 1024], in_=tp16)
                nc.sync.dma_start(out=w_out[h, qb * 128:(qb + 1) * 128, :],
                                  in_=wrow)

        # ================= o_proj =================
        wo_t = [big_tile([128, 2, E], F32R, t, f"woh{i}")
                for i, t in enumerate(("tE", "tF"))]
        for i in range(2):
            nc.sync.dma_start(
                out=wo_t[i],
                in_=wo.rearrange("(n p) e -> p n e", p=128)[:, i * 2:(i + 1) * 2])
        aT_all = [big_tile([128, 2, S], F32R, t, f"aTall{i}")
                  for i, t in enumerate(("tA", "tB"))]
        for i in range(2):
            nc.sync.dma_start(
                out=aT_all[i],
                in_=attnT_s.rearrange("h d s -> d h s")[:, i * 2:(i + 1) * 2])
        for tc_i in range(NKC):
            tsl = slice(tc_i * 128, (tc_i + 1) * 128)
            for es in range(4):
                esl = slice(es * 512, (es + 1) * 512)
                op_ps = ps_a([128, 512], f"opps{tc_i}{es}")
                for hh in range(HPC):
                    nc.tensor.matmul(op_ps, aT_all[hh // 2][:, hh % 2, tsl],
                                     wo_t[hh // 2][:, hh % 2, esl],
                                     start=(hh == 0), stop=(hh == HPC - 1))
                o_sb = work.tile([128, 512], F32, tag="osb", bufs=2, name="o_sb")
                nc.scalar.copy(out=o_sb, in_=op_ps)
                nc.sync.dma_start(out=out_p[tsl, esl], in_=o_sb)

    nc.compile()
    return nc


def _get_nc():
    global _NC_CACHE
    if _NC_CACHE is None:
        _NC_CACHE = build_kernel()
    return _NC_CACHE


def kernel(x, sin, cos, mask, Wq, bq, Wk, bk, Wv, bv, Wo, bo):
    x = np.asarray(x, dtype=np.float32)
    sin = np.asarray(sin, dtype=np.float32)
    cos = np.asarray(cos, dtype=np.float32)
    mask = np.asarray(mask)
    Wq = np.asarray(Wq, dtype=np.float32)
    Wk = np.asarray(Wk, dtype=np.float32)
    Wv = np.asarray(Wv, dtype=np.float32)
    Wo = np.asarray(Wo, dtype=np.float32)
    bq_ = np.asarray(bq, dtype=np.float32)
    bk_ = np.asarray(bk, dtype=np.float32)
    bv_ = np.asarray(bv, dtype=np.float32)
    bo_ = np.asarray(bo, dtype=np.float32)

    nc = _get_nc()

    in_maps = []
    for c in range(8):
        b, g = c // HPC, c % HPC
        hsl = slice(g * HPC, (g + 1) * HPC)          # heads 4g..4g+3
        csl = slice(g * DL, (g + 1) * DL)            # E-columns for those heads
        mm1T = (mask[b].T.astype(np.float32) - 1.0).astype(ml_dtypes.float8_e5m2)
        in_maps.append(dict(
            xT=np.ascontiguousarray(x[b].T),
            wq=np.ascontiguousarray(Wq[:, csl]),
            wk=np.ascontiguousarray(Wk[:, csl]),
            wv=np.ascontiguousarray(Wv[:, csl]),
            wo=np.ascontiguousarray(Wo[csl, :]),
            bq=np.ascontiguousarray(bq_[csl])[None, :],
            bk=np.ascontiguousarray(bk_[csl])[None, :],
            bv=np.ascontiguousarray(bv_[csl])[None, :],
            sinT=np.ascontiguousarray(np.swapaxes(sin[0, hsl], 1, 2)),
            cosT=np.ascontiguousarray(np.swapaxes(cos[0, hsl], 1, 2)),
            mm1T=np.ascontiguousarray(mm1T),
        ))

    res = run_bass_kernel_spmd(nc, in_maps, list(range(8)))

    out = np.zeros((B, S, E), dtype=np.float32)
    attw = np.empty((B, H, S, S), dtype=np.float32)
    for c in range(8):
        b, g = c // HPC, c % HPC
        out[b] += res.results[c]["out_p"]
        attw[b, g * HPC:(g + 1) * HPC] = res.results[c]["w_out"]
    out += bo_[None, None, :]
    return out, attw


# revision 19
# speedup vs baseline: 24043.6315x; 24043.6315x over previous
"""TRN2 Bass kernel for nn_MLA_87892210746097.

MHA with RoPE, double softmax, o_proj. B=2, S=2048, E=2048, H=16, D=128.
Sharding: 8 cores = 2 batches x 4 head-groups (4 heads each); the o_proj
all-reduce (4 partial sums per batch) and bias add happen on the host after
the gather, as does the head-axis concat of the attention weights.
Returns (out, attention_weights) like the reference.
"""
import sys
import numpy as np
import ml_dtypes
from contextlib import ExitStack

sys.path.insert(0, "/opt/trn_rl_repo")

import concourse.bass as bass
import concourse.mybir as mybir
import concourse.tile as tile
from concourse import bacc
from concourse.bass_utils import run_bass_kernel_spmd

F32 = mybir.dt.float32
F32R = mybir.dt.float32r
F16 = mybir.dt.float16
FP8 = mybir.dt.float8e5
AF = mybir.ActivationFunctionType
ALU = mybir.AluOpType

B, S, E = 2, 2048, 2048
H, D = 16, 128
HPC = 4            # heads per core
DL = HPC * D       # 512: local E-slice per core
NKC = S // 128     # 16 chunks of 128
NTS = S // 512     # 4 slices of 512
NEC = E // 128     # 16 e-chunks
SCALE = 1.0 / float(np.sqrt(D))
BIG = 57344.0      # fp8e5-exact large value for the mask seed

_NC_CACHE = None


def _swap_matrix():
    """lhsT for rot = M @ q^T where M is the minus_swap permutation."""
    Mt = np.zeros((D, D), dtype=np.float32)
    for i in range(D // 2):
        Mt[2 * i + 1, 2 * i] = -1.0   # out[2i]   = -q[2i+1]
        Mt[2 * i, 2 * i + 1] = 1.0    # out[2i+1] =  q[2i]
    return Mt


def build_kernel():
    nc = bacc.Bacc(None, target_bir_lowering=False)

    # ---------------- I/O ----------------
    xT = nc.declare_dram_parameter("xT", [E, S], F32R, isOutput=False)
    wq = nc.declare_dram_parameter("wq", [E, DL], F32R, isOutput=False)
    wk = nc.declare_dram_parameter("wk", [E, DL], F32R, isOutput=False)
    wv = nc.declare_dram_parameter("wv", [E, DL], F32R, isOutput=False)
    wo = nc.declare_dram_parameter("wo", [DL, E], F32R, isOutput=False)
    bq = nc.declare_dram_parameter("bq", [1, DL], F32, isOutput=False)
    bk = nc.declare_dram_parameter("bk", [1, DL], F32, isOutput=False)
    bv = nc.declare_dram_parameter("bv", [1, DL], F32, isOutput=False)
    sinT = nc.declare_dram_parameter("sinT", [HPC, D, S], F32, isOutput=False)
    cosT = nc.declare_dram_parameter("cosT", [HPC, D, S], F32, isOutput=False)
    # host-prepped (mask.T - 1) in fp8e5: 0 where visible, -1 where masked
    mm1T = nc.declare_dram_parameter("mm1T", [S, S], FP8, isOutput=False)

    out_p = nc.declare_dram_parameter("out_p", [S, E], F32, isOutput=True)
    w_out = nc.declare_dram_parameter("w_out", [HPC, S, S], F32, isOutput=True)

    # ---------------- constants ----------------
    swapM = nc.inline_tensor(_swap_matrix(), name="swapM")
    seedI = nc.inline_tensor(
        (np.eye(128) * BIG).astype(np.float16), name="seedI")
    ones128 = nc.inline_tensor(np.ones((128, 1), dtype=np.float16), name="ones128")
    identF = nc.inline_tensor(np.eye(128, dtype=np.float32), name="identF")
    ident16 = nc.inline_tensor(np.eye(128).astype(np.float16), name="ident16")

    # ---------------- scratch ----------------
    qT_s = nc.dram_tensor("qT_s", [HPC, D, S], F32R)
    kT_s = nc.dram_tensor("kT_s", [HPC, D, S], F32R)
    v_s = nc.dram_tensor("v_s", [S, DL], F16)
    attnT_s = nc.dram_tensor("attnT_s", [HPC, D, S], F32R)
    r_s = nc.dram_tensor("r_s", [1, S], F32)
    r2_s = nc.dram_tensor("r2_s", [1, S], F32)

    with ExitStack() as ctx:
        tc = ctx.enter_context(tile.TileContext(nc))
        consts = ctx.enter_context(tc.tile_pool(name="consts", bufs=1))
        big = ctx.enter_context(tc.tile_pool(name="big", bufs=1))
        work = ctx.enter_context(tc.tile_pool(name="work", bufs=1))
        reps = ctx.enter_context(tc.tile_pool(name="reps", bufs=1))
        ps = ctx.enter_context(tc.tile_pool(name="ps", bufs=1, space="PSUM"))

        def ps_a(shape, name):
            return ps.tile(shape, F32, tag="a", bufs=2, name=name)

        def ps_b(shape, name):
            return ps.tile(shape, F32, tag="b", bufs=2, name=name)

        def ps_d(shape, name):
            return ps.tile(shape, F32, tag="d", bufs=1, name=name)

        swap_sb = consts.tile([D, D], F32R)
        nc.sync.dma_start(out=swap_sb, in_=swapM[:].bitcast(F32R))
        seed16 = consts.tile([128, 128], F16)
        nc.sync.dma_start(out=seed16, in_=seedI[:])
        seed_sb = consts.tile([128, 128], FP8)
        nc.vector.tensor_copy(out=seed_sb, in_=seed16)
        ones128_sb = consts.tile([128, 1], F16)
        nc.sync.dma_start(out=ones128_sb, in_=ones128[:])
        ident_sb = consts.tile([128, 128], F32)
        nc.sync.dma_start(out=ident_sb, in_=identF[:])
        ident16_sb = consts.tile([128, 128], F16)
        nc.sync.dma_start(out=ident16_sb, in_=ident16[:])
        # per-partition bias views: (p, c) <- b[0, c*128 + p]
        bqt = consts.tile([128, HPC], F32)
        nc.sync.dma_start(out=bqt, in_=bq[0].rearrange("(c p) -> p c", p=128))
        bkt = consts.tile([128, HPC], F32)
        nc.sync.dma_start(out=bkt, in_=bk[0].rearrange("(c p) -> p c", p=128))
        # bv broadcast to 128 partitions (free-axis bias add on V)
        bv_rep = consts.tile([128, DL], F32)
        bvap = bv[:]
        nc.gpsimd.dma_start(
            out=bv_rep,
            in_=bass.AP(tensor=bvap.tensor, offset=bvap.offset,
                        ap=[[0, 128], bvap.ap[-1]]))

        # ============ big slab tags (8 x 16KB/partition) ============
        # tA/tB: wq halves -> pw slabs 0/1 -> aT_all halves
        # tC/tD: wk halves -> pw slabs 2/3
        # tE/tF: wv halves -> wo halves
        # tG/tH: x-slice halves
        # sq: qT -> vloc ; sk: kT -> values
        def big_tile(shape, dtype, tag, name):
            return big.tile(shape, dtype, tag=tag, bufs=1, name=name)

        wq_t = [big_tile([128, 8, DL], F32R, t, f"wqh{i}")
                for i, t in enumerate(("tA", "tB"))]
        wk_t = [big_tile([128, 8, DL], F32R, t, f"wkh{i}")
                for i, t in enumerate(("tC", "tD"))]
        wv_t = [big_tile([128, 8, DL], F32R, t, f"wvh{i}")
                for i, t in enumerate(("tE", "tF"))]
        for i in range(2):
            half = slice(i * 8, (i + 1) * 8)
            nc.sync.dma_start(
                out=wq_t[i], in_=wq.rearrange("(n p) d -> p n d", p=128)[:, half])
            nc.sync.dma_start(
                out=wk_t[i], in_=wk.rearrange("(n p) d -> p n d", p=128)[:, half])
            nc.sync.dma_start(
                out=wv_t[i], in_=wv.rearrange("(n p) d -> p n d", p=128)[:, half])

        def wslab(tiles, ec):
            return tiles[ec // 8][:, ec % 8, :]

        # ================= projection phase =================
        for ts_i in range(NTS):
            tsl = slice(ts_i * 512, (ts_i + 1) * 512)
            x_t = [big_tile([128, 8, 512], F32R, t, f"xh{i}_{ts_i}")
                   for i, t in enumerate(("tG", "tH"))]
            for i in range(2):
                half = slice(i * 8, (i + 1) * 8)
                nc.sync.dma_start(
                    out=x_t[i],
                    in_=xT.rearrange("(n p) s -> p n s", p=128)[:, half, tsl])

            # --- q^T and k^T (d on partitions) + RoPE ---
            for w_t, bias_t, dst in ((wq_t, bqt, qT_s), (wk_t, bkt, kT_s)):
                for oc in range(HPC):  # head == o-chunk of 128
                    pj_ps = ps_a([128, 512], f"pj{ts_i}{oc}")
                    for ec in range(NEC):
                        nc.tensor.matmul(
                            pj_ps, wslab(w_t, ec)[:, oc * 128:(oc + 1) * 128],
                            wslab(x_t, ec), start=(ec == 0), stop=(ec == NEC - 1))
                    qraw = work.tile([128, 512], F32, tag="qraw", name="qraw")
                    nc.scalar.activation(out=qraw, in_=pj_ps, func=AF.Identity,
                                         bias=bias_t[:, oc:oc + 1], scale=1.0)
                    qraw_r = work.tile([128, 512], F32R, tag="qrawr", name="qraw_r")
                    nc.vector.tensor_copy(out=qraw_r, in_=qraw)
                    rot_ps = ps_b([128, 512], f"rot{ts_i}{oc}")
                    nc.tensor.matmul(rot_ps, swap_sb, qraw_r, start=True, stop=True)
                    sin_t = work.tile([128, 512], F32, tag="sin", name="sin_t")
                    cos_t = work.tile([128, 512], F32, tag="cos", name="cos_t")
                    nc.sync.dma_start(out=sin_t, in_=sinT[oc, :, tsl])
                    nc.sync.dma_start(out=cos_t, in_=cosT[oc, :, tsl])
                    t1 = work.tile([128, 512], F32, tag="t1", name="t1")
                    nc.vector.tensor_tensor(t1, qraw, cos_t, ALU.mult)
                    t2 = work.tile([128, 512], F32, tag="t2", name="t2")
                    nc.vector.tensor_tensor(t2, rot_ps, sin_t, ALU.mult)
                    qrot = work.tile([128, 512], F32R, tag="qrawr", name="qrot")
                    nc.vector.tensor_tensor(qrot, t1, t2, ALU.add)
                    nc.sync.dma_start(out=dst[oc, :, tsl], in_=qrot)

            # --- V (tokens on partitions) ---
            for tci in range(4):
                tchunk = ts_i * 4 + tci
                v_ps = ps_a([128, DL], f"vps{tchunk}")
                for ec in range(NEC):
                    nc.tensor.matmul(
                        v_ps, wslab(x_t, ec)[:, tci * 128:(tci + 1) * 128],
                        wslab(wv_t, ec), start=(ec == 0), stop=(ec == NEC - 1))
                v_sb = work.tile([128, DL], F16, tag="vsb", bufs=2, name="v_sb")
                nc.vector.tensor_tensor(v_sb, v_ps, bv_rep, ALU.add)
                nc.sync.dma_start(out=v_s[tchunk * 128:(tchunk + 1) * 128, :],
                                  in_=v_sb)

        # ================= attention per head =================
        for h in range(HPC):
            qT_sb = big_tile([D, S], F32R, "sq", f"qT{h}")
            kT_sb = big_tile([D, S], F32R, "sk", f"kT{h}")
            nc.sync.dma_start(out=qT_sb, in_=qT_s[h])
            nc.sync.dma_start(out=kT_sb, in_=kT_s[h])
            pw_t = [big_tile([128, 4, S], F16, t, f"pw{i}_{h}")
                    for i, t in enumerate(("tA", "tB", "tC", "tD"))]

            def pw(kc):
                return pw_t[kc // 4][:, kc % 4, :]

            # ---- pass 1: logits -> exp -> P^T, rowsum r (partition dir) ----
            r_q = [ps.tile([1, 512], F32, tag=t, bufs=2, name=f"rq{j}_{h}")
                   for j, t in enumerate(("a", "a", "b", "b"))]
            for kc in range(NKC):
                mk = work.tile([128, S], FP8, tag="mk", bufs=2, name="mk")
                nc.sync.dma_start(out=mk, in_=mm1T[kc * 128:(kc + 1) * 128, :])
                for hf in range(2):
                    hsl_ = slice(hf * 1024, (hf + 1) * 1024)
                    lg_ps = ps.tile([128, 1024], F32, tag="d", bufs=2,
                                    name=f"lg{h}{kc}{hf}")
                    for j in range(2):
                        jsl = slice(hf * 1024 + j * 512, hf * 1024 + (j + 1) * 512)
                        jloc = slice(j * 512, (j + 1) * 512)
                        nc.tensor.matmul(lg_ps[:, jloc], seed_sb, mk[:, jsl],
                                         start=True, stop=False)
                        nc.tensor.matmul(lg_ps[:, jloc],
                                         kT_sb[:, kc * 128:(kc + 1) * 128],
                                         qT_sb[:, jsl], start=False, stop=True)
                    nc.scalar.activation(out=pw(kc)[:, hsl_], in_=lg_ps,
                                         func=AF.Exp, bias=0.0, scale=SCALE)
                for j in range(NTS):
                    jsl = slice(j * 512, (j + 1) * 512)
                    nc.tensor.matmul(r_q[j], ones128_sb, pw(kc)[:, jsl],
                                     start=(kc == 0), stop=(kc == NKC - 1))

            # 1/r -> dram roundtrip; fp16 bcast rep + per-partition chunks
            rrecf = reps.tile([1, S], F32, tag="rrecf", name="rrecf")
            for j in range(NTS):
                nc.vector.reciprocal(out=rrecf[:, j * 512:(j + 1) * 512],
                                     in_=r_q[j])
            nc.sync.dma_start(out=r_s[:], in_=rrecf)
            rpp = reps.tile([128, NKC], F32, tag="rpp", name="rpp")
            nc.sync.dma_start(out=rpp, in_=r_s[0].rearrange("(c p) -> p c", p=128))
            rrep16 = reps.tile([128, S], F16, tag="rrep16", name="rrep16")
            rsap = r_s[:]
            nc.gpsimd.dma_start(
                out=rrep16,
                in_=bass.AP(tensor=rsap.tensor, offset=rsap.offset,
                            ap=[[0, 128], rsap.ap[-1]]))

            # ---- pass 2: values accum, A^T = P^T/r, exp2 -> Wun, rowsum2 ----
            vloc = big_tile([128, NKC, D], F16, "sq", f"vloc{h}")
            nc.sync.dma_start(
                out=vloc,
                in_=v_s.rearrange("(n p) d -> p n d", p=128)[:, :, h * D:(h + 1) * D])
            valT_h = [ps.tile([128, 1024], F32, tag="d", bufs=2,
                              name=f"valT{h}{i}") for i in range(2)]
            r2_q = [ps.tile([1, 512], F32, tag=t, bufs=2, name=f"r2q{j}_{h}")
                    for j, t in enumerate(("a", "a", "b", "b"))]
            for kc in range(NKC):
                for j in range(NTS):
                    jsl = slice(j * 512, (j + 1) * 512)
                    nc.tensor.matmul(
                        valT_h[j // 2][:, (j % 2) * 512:(j % 2 + 1) * 512],
                        vloc[:, kc, :], pw(kc)[:, jsl],
                        start=(kc == 0), stop=(kc == NKC - 1))
                aT = work.tile([128, S], F16, tag="aT", bufs=2, name="aT")
                nc.vector.tensor_tensor(aT, pw(kc), rrep16, ALU.mult)
                nc.scalar.activation(out=pw(kc), in_=aT,
                                     func=AF.Exp, bias=0.0, scale=1.0)
                for j in range(NTS):
                    jsl = slice(j * 512, (j + 1) * 512)
                    nc.tensor.matmul(r2_q[j], ones128_sb, pw(kc)[:, jsl],
                                     start=(kc == 0), stop=(kc == NKC - 1))

            # values^T = valT/r (free-axis scale) -> fp16, then PE-transpose
            # to (s, d) layout for the pass-3 stationary
            valuesT = work.tile([D, S], F16, tag="vT", name="valuesT")
            for i in range(2):
                nc.vector.tensor_tensor(valuesT[:, i * 1024:(i + 1) * 1024],
                                        valT_h[i], rrep16[:, i * 1024:(i + 1) * 1024],
                                        ALU.mult)
            values = big_tile([128, NKC, D], F16, "sk", f"values{h}")
            for sc in range(NKC):
                vt_ps = ps_a([128, 128], f"vt{h}{sc}") if sc % 2 == 0 else \
                    ps_b([128, 128], f"vt{h}{sc}")
                vt_ps = vt_ps.bitcast(F16)[:, 0:128]
                nc.tensor.transpose(
                    vt_ps, valuesT[:, sc * 128:(sc + 1) * 128], ident16_sb)
                nc.scalar.copy(out=values[:, sc, :], in_=vt_ps)

            # 1/r2 -> dram roundtrip -> fp16 broadcast rep
            r2recf = reps.tile([1, S], F32, tag="rrecf", name="r2recf")
            for j in range(NTS):
                nc.vector.reciprocal(out=r2recf[:, j * 512:(j + 1) * 512],
                                     in_=r2_q[j])
            nc.sync.dma_start(out=r2_s[:], in_=r2recf)
            r2rep16 = reps.tile([128, S], F16, tag="r2rep16", name="r2rep16")
            r2ap = r2_s[:]
            nc.gpsimd.dma_start(
                out=r2rep16,
                in_=bass.AP(tensor=r2ap.tensor, offset=r2ap.offset,
                            ap=[[0, 128], r2ap.ap[-1]]))

            # ---- pass 3a: attn_un^T accum over k, scale by 1/r2 ----
            at_h = [ps.tile([128, 1024], F32, tag="d", bufs=2,
                            name=f"atps{h}{i}") for i in range(2)]
            for kc in range(NKC):
                for j in range(NTS):
                    jsl = slice(j * 512, (j + 1) * 512)
                    nc.tensor.matmul(
                        at_h[j // 2][:, (j % 2) * 512:(j % 2 + 1) * 512],
                        values[:, kc, :], pw(kc)[:, jsl],
                        start=(kc == 0), stop=(kc == NKC - 1))
            for j in range(NTS):
                jsl = slice(j * 512, (j + 1) * 512)
                attnT = work.tile([128, 512], F32R, tag="osb", bufs=2,
                                  name="attnT")
                nc.vector.tensor_tensor(
                    attnT, at_h[j // 2][:, (j % 2) * 512:(j % 2 + 1) * 512],
                    r2rep16[:, jsl], ALU.mult)
                nc.sync.dma_start(out=attnT_s[h][:, jsl], in_=attnT)

            # ---- pass 3b: W output: scale, transpose, write (q,k) rows ----
            # scale slab in place (W^T = Wun^T / r2), row-wise fp16
            for kc in range(NKC):
                nc.vector.tensor_tensor(pw(kc), pw(kc), r2rep16, ALU.mult)
            for qb in range(NKC):
                wrow = work.tile([128, S], F32, tag="qraw", name="wrow")
                for hf in range(2):
                    tp_ps = ps.tile([128, 1024], F32, tag="d", bufs=2,
                                    name=f"tp{h}{qb}{hf}")
                    tp16 = tp_ps.bitcast(F16)[:, 0:1024]
                    for kk in range(8):
                        kc = hf * 8 + kk
                        nc.tensor.matmul(
                            tp16[:, kk * 128:(kk + 1) * 128],
                            pw(kc)[:, qb * 128:(qb + 1) * 128], ident16_sb,
                            is_transpose=True,
                            start=(kk == 0), stop=(kk == 7))
                    nc.any.tensor_copy(
                        out=wrow[:, hf * 1024:(hf + 1) * 1024], in_=tp16)
                eng = nc.sync if qb % 2 == 0 else nc.scalar
                eng.dma_start(out=w_out[h, qb * 128:(qb + 1) * 128, :],
                              in_=wrow)

        # ================= o_proj =================
        wo_t = [big_tile([128, 2, E], F32R, t, f"woh{i}")
                for i, t in enumerate(("tE", "tF"))]
        for i in range(2):
            nc.sync.dma_start(
                out=wo_t[i],
                in_=wo.rearrange("(n p) e -> p n e", p=128)[:, i * 2:(i + 1) * 2])
        aT_all = [big_tile([128, 2, S], F32R, t, f"aTall{i}")
                  for i, t in enumerate(("tA", "tB"))]
        for i in range(2):
            nc.sync.dma_start(
                out=aT_all[i],
                in_=attnT_s.rearrange("h d s -> d h s")[:, i * 2:(i + 1) * 2])
        for tc_i in range(NKC):
            tsl = slice(tc_i * 128, (tc_i + 1) * 128)
            for es in range(4):
                esl = slice(es * 512, (es + 1) * 512)
                op_ps = ps_a([128, 512], f"opps{tc_i}{es}")
                for hh in range(HPC):
                    nc.tensor.matmul(op_ps, aT_all[hh // 2][:, hh % 2, tsl],
                                     wo_t[hh // 2][:, hh % 2, esl],
                                     start=(hh == 0), stop=(hh == HPC - 1))
                o_sb = work.tile([128, 512], F32, tag="osb", bufs=2, name="o_sb")
                nc.scalar.copy(out=o_sb, in_=op_ps)
                eng = nc.sync if es % 2 == 0 else nc.scalar
                eng.dma_start(out=out_p[tsl, esl], in_=o_sb)

    nc.compile()
    return nc


def _get_nc():
    global _NC_CACHE
    if _NC_CACHE is None:
        _NC_CACHE = build_kernel()
    return _NC_CACHE


def kernel(x, sin, cos, mask, Wq, bq, Wk, bk, Wv, bv, Wo, bo):
    x = np.asarray(x, dtype=np.float32)
    sin = np.asarray(sin, dtype=np.float32)
    cos = np.asarray(cos, dtype=np.float32)
    mask = np.asarray(mask)
    Wq = np.asarray(Wq, dtype=np.float32)
    Wk = np.asarray(Wk, dtype=np.float32)
    Wv = np.asarray(Wv, dtype=np.float32)
    Wo = np.asarray(Wo, dtype=np.float32)
    bq_ = np.asarray(bq, dtype=np.float32)
    bk_ = np.asarray(bk, dtype=np.float32)
    bv_ = np.asarray(bv, dtype=np.float32)
    bo_ = np.asarray(bo, dtype=np.float32)

    nc = _get_nc()

    in_maps = []
    for c in range(8):
        b, g = c // HPC, c % HPC
        hsl = slice(g * HPC, (g + 1) * HPC)          # heads 4g..4g+3
        csl = slice(g * DL, (g + 1) * DL)            # E-columns for those heads
        mm1T = (mask[b].T.astype(np.float32) - 1.0).astype(ml_dtypes.float8_e5m2)
        in_maps.append(dict(
            xT=np.ascontiguousarray(x[b].T),
            wq=np.ascontiguousarray(Wq[:, csl]),
            wk=np.ascontiguousarray(Wk[:, csl]),
            wv=np.ascontiguousarray(Wv[:, csl]),
            wo=np.ascontiguousarray(Wo[csl, :]),
            bq=np.ascontiguousarray(bq_[csl])[None, :],
            bk=np.ascontiguousarray(bk_[csl])[None, :],
            bv=np.ascontiguousarray(bv_[csl])[None, :],
            sinT=np.ascontiguousarray(np.swapaxes(sin[0, hsl], 1, 2)),
            cosT=np.ascontiguousarray(np.swapaxes(cos[0, hsl], 1, 2)),
            mm1T=np.ascontiguousarray(mm1T),
        ))

    res = run_bass_kernel_spmd(nc, in_maps, list(range(8)))

    out = np.zeros((B, S, E), dtype=np.float32)
    attw = np.empty((B, H, S, S), dtype=np.float32)
    for c in range(8):
        b, g = c // HPC, c % HPC
        out[b] += res.results[c]["out_p"]
        attw[b, g * HPC:(g + 1) * HPC] = res.results[c]["w_out"]
    out += bo_[None, None, :]
    return out, attw
